# revision 5
# baseline (speedup 1.0000x reference)
"""Trainium2 Bass kernel for nn_LGE_Exp: out[u,site] = expm(i*sum_m w[u,m]*tlh(W[m,site])) @ U[u,site].

Self-contained: hardcodes shapes (4,16,16,16,32,3,3) and 8-core site sharding.
Math: per-site 3x3 complex expm via uniform scaling (2^-s), degree-9 Taylor in
Paterson-Stockmeyer form (powers B, B2), s squarings, then M @ U.
All heavy math on-device as f32 planes; host only packs/unpacks layouts.
"""
import math
import sys

import numpy as np

sys.path.insert(0, '/opt/trn_rl_repo')

NDIR = 4
NF = 4
LAT = (16, 16, 16, 32)
V = 16 * 16 * 16 * 32          # 131072 sites
NCORES = 8
SPC = V // NCORES              # 16384 sites per core
P = 128                        # partitions
FS = SPC // P                  # 128 site columns per plane
FD = NDIR * FS                 # 512 site-dir columns
THETA = 0.9                    # max ||B||_F after scaling
DEG = 9                        # Taylor degree (PS q=2 -> 1 + 4 horner matmuls)

_CACHE = {}


def _q(r, i, j):
    return r * 9 + i * 3 + j


def _build_program(vr, vi, s):
    """vr, vi: [4u,4m] float effective weights (v = sigma*i*w/2). s: squarings."""
    import concourse.bacc as bacc
    import concourse.tile as tile
    import concourse.mybir as mybir

    dt = mybir.dt.float32
    op = mybir.AluOpType
    nc = bacc.Bacc("TRN2", target_bir_lowering=False)

    Win = nc.dram_tensor("Win", [P, NF, 18, FS], dt, kind="ExternalInput")
    Uin = nc.dram_tensor("Uin", [P, 18, NDIR, FS], dt, kind="ExternalInput")
    Out = nc.dram_tensor("Out", [P, 18, NDIR, FS], dt, kind="ExternalOutput")

    cofs = [1.0 / math.factorial(k) for k in range(DEG + 1)]

    with tile.TileContext(nc) as tc:
        import contextlib
        ctx = contextlib.ExitStack()
        with ctx:
            poolB = ctx.enter_context(tc.tile_pool(name="b", bufs=1))
            poolPB2 = ctx.enter_context(tc.tile_pool(name="pb2", bufs=1))
            poolH1 = ctx.enter_context(tc.tile_pool(name="h1", bufs=1))
            ctx_early = contextlib.ExitStack()
            poolW = ctx_early.enter_context(tc.tile_pool(name="w", bufs=2))
            poolP = ctx_early.enter_context(tc.tile_pool(name="p", bufs=1))
            poolHt = ctx_early.enter_context(tc.tile_pool(name="ht", bufs=1))
            tmpa = ctx_early.enter_context(tc.tile_pool(name="tmpa", bufs=8))

            def t512():
                return tmps.tile([P, FD], dt, tag="t512", name="t512")[:]

            def t128():
                return tmpa.tile([P, FS], dt, tag="t128", name="t128")[:]

            # ---- load W per m, build P planes (P = 2*tlh, hermitian) ----
            # P planes per m: pre[i<=j] (6), pim[i<j] (3) -> 9 planes [P, FS]
            Pt = poolP.tile([P, NF, 9, FS], dt, tag="pp", name="Pt")
            # P-plane order: pre(00,01,02,11,12,22), pim(01,02,12)
            pre_ix = {(0, 0): 0, (0, 1): 1, (0, 2): 2, (1, 1): 3, (1, 2): 4, (2, 2): 5}
            pim_ix = {(0, 1): 6, (0, 2): 7, (1, 2): 8}
            for m in range(NF):
                wm = poolW.tile([P, 18, FS], dt, tag="wm", name="wm")
                nc.sync.dma_start(out=wm[:], in_=Win[:, m])

                def wre(i, j):
                    return wm[:, _q(0, i, j)]

                def wim(i, j):
                    return wm[:, _q(1, i, j)]

                # c2c = (2/3)*(Wim00+Wim11+Wim22)
                c1 = t128()
                nc.vector.tensor_tensor(c1, wim(0, 0), wim(1, 1), op=op.add)
                c2b = t128()
                nc.vector.tensor_scalar(c2b, c1, 2.0 / 3.0, None, op0=op.mult)
                c2c = t128()
                nc.vector.scalar_tensor_tensor(c2c, wim(2, 2), 2.0 / 3.0, c2b,
                                               op0=op.mult, op1=op.add)
                for i in range(3):
                    # pre diag: 2*Wim_ii - (2/3)c
                    nc.vector.scalar_tensor_tensor(
                        Pt[:, m, pre_ix[(i, i)]], wim(i, i), 2.0, c2c,
                        op0=op.mult, op1=op.subtract)
                for (i, j), ix in pre_ix.items():
                    if i != j:
                        nc.vector.tensor_tensor(Pt[:, m, ix], wim(i, j), wim(j, i),
                                                op=op.add)
                for (i, j), ix in pim_ix.items():
                    nc.vector.tensor_tensor(Pt[:, m, ix], wre(j, i), wre(i, j),
                                            op=op.subtract)

            # ---- assemble B[u] = sum_m (vr+i*vi)_um * P_m  (B: [P,18,FD]) ----
            Bt = poolB.tile([P, 18, FD], dt, tag="bu", name="Bt")

            def bpl(r, i, j):
                return Bt[:, _q(r, i, j)]

            for u in range(NDIR):
                Ht = poolHt.tile([P, 18, FS], dt, tag="htmp", name="Ht")
                # H planes: H1 (6: vr*pre), H2 (6: vi*pre), H3 (3: vr*pim), H4 (3: vi*pim)
                specs = []
                for (pair, ix) in pre_ix.items():
                    specs.append((ix, ix, vr[u]))          # H1 at 0..5
                for (pair, ix) in pre_ix.items():
                    specs.append((6 + ix, ix, vi[u]))      # H2 at 6..11
                for (pair, ix) in pim_ix.items():
                    specs.append((12 + (ix - 6), ix, vr[u]))  # H3 at 12..14
                for (pair, ix) in pim_ix.items():
                    specs.append((15 + (ix - 6), ix, vi[u]))  # H4 at 15..17
                for hix, pix, coef in specs:
                    a = t128()
                    nc.vector.tensor_scalar(a, Pt[:, 0, pix], float(coef[0]), None,
                                            op0=op.mult)
                    b = t128()
                    nc.vector.scalar_tensor_tensor(b, Pt[:, 1, pix], float(coef[1]),
                                                   a, op0=op.mult, op1=op.add)
                    a2 = t128()
                    nc.vector.scalar_tensor_tensor(a2, Pt[:, 2, pix], float(coef[2]),
                                                   b, op0=op.mult, op1=op.add)
                    nc.vector.scalar_tensor_tensor(Ht[:, hix], Pt[:, 3, pix],
                                                   float(coef[3]), a2,
                                                   op0=op.mult, op1=op.add)

                def h1(i, j):
                    return Ht[:, pre_ix[(min(i, j), max(i, j))]]

                def h2(i, j):
                    return Ht[:, 6 + pre_ix[(min(i, j), max(i, j))]]

                def h3(i, j):
                    return Ht[:, 12 + pim_ix[(min(i, j), max(i, j))] - 6]

                def h4(i, j):
                    return Ht[:, 15 + pim_ix[(min(i, j), max(i, j))] - 6]

                def bblk(r, i, j):
                    return Bt[:, _q(r, i, j), u * FS:(u + 1) * FS]

                for i in range(3):
                    for j in range(3):
                        if i == j:
                            nc.scalar.copy(bblk(0, i, j), h1(i, j))
                            nc.scalar.copy(bblk(1, i, j), h2(i, j))
                        elif i < j:
                            nc.vector.tensor_tensor(bblk(0, i, j), h1(i, j), h4(i, j),
                                                    op=op.subtract)
                            nc.vector.tensor_tensor(bblk(1, i, j), h2(i, j), h3(i, j),
                                                    op=op.add)
                        else:
                            nc.vector.tensor_tensor(bblk(0, i, j), h1(i, j), h4(i, j),
                                                    op=op.add)
                            nc.vector.tensor_tensor(bblk(1, i, j), h2(i, j), h3(i, j),
                                                    op=op.subtract)

            # ---- complex 3x3 matmul helper over planes ----
            gct = [0]

            def pick_eng(i, j, r):
                # off-diag entries rotate to gpsimd ~1/3 of instructions
                if (i, j) in ((0, 1), (1, 2), (2, 0)):
                    return nc.gpsimd
                return nc.vector

            def cmatmul(cpl, apl, bpl_, fold=None, foldB=None):
                """cpl/apl/bpl_: fn(r,i,j)->AP. fold=(c1,c0): C += c1*foldB + c0*I."""
                for i in range(3):
                    for j in range(3):
                        eng = pick_eng(i, j, 0)
                        is_g = eng is nc.gpsimd
                        for r in (0, 1):
                            if r == 0:
                                combos = [((0, 0), (1, 1), op.subtract)]
                            else:
                                combos = [((0, 1), (1, 0), op.add)]
                            (ra, rb), (rc, rd), cop = combos[0]
                            ps = []
                            for k in range(3):
                                m1 = t512()
                                eng.tensor_tensor(m1, apl(ra, i, k), bpl_(rb, k, j),
                                                  op=op.mult)
                                m2 = t512()
                                eng.tensor_tensor(m2, apl(rc, i, k), bpl_(rd, k, j),
                                                  op=op.mult)
                                pk = t512()
                                eng.tensor_tensor(pk, m1, m2, op=cop)
                                ps.append(pk)
                            t4 = t512()
                            eng.tensor_tensor(t4, ps[0], ps[1], op=op.add)
                            dest = cpl(r, i, j)
                            if fold is None:
                                eng.tensor_tensor(dest, ps[2], t4, op=op.add)
                            else:
                                c1, c0 = fold
                                t5 = t512()
                                eng.tensor_tensor(t5, ps[2], t4, op=op.add)
                                fb = foldB(r, i, j)
                                if r == 0 and i == j:
                                    t6 = t512()
                                    nc.vector.tensor_scalar(t6, fb, c1, c0,
                                                            op0=op.mult, op1=op.add)
                                    nc.vector.tensor_tensor(dest, t6, t5, op=op.add)
                                elif is_g:
                                    t6 = t512()
                                    eng.tensor_scalar(t6, fb, c1, None, op0=op.mult)
                                    eng.tensor_tensor(dest, t6, t5, op=op.add)
                                else:
                                    eng.scalar_tensor_tensor(dest, fb, c1, t5,
                                                             op0=op.mult, op1=op.add)

            # ---- early pools done; free them, open late pools ----
            ctx_early.close()
            poolH2 = ctx.enter_context(tc.tile_pool(name="h2", bufs=1))
            tmps = ctx.enter_context(tc.tile_pool(name="tmps", bufs=12))

            # ---- B2 = B*B ----
            B2t = poolPB2.tile([P, 18, FD], dt, tag="pb2", name="B2t")

            def b2pl(r, i, j):
                return B2t[:, _q(r, i, j)]

            cmatmul(b2pl, bpl, bpl)

            # ---- Taylor deg 9, PS q=2: H = G4; H = H*B2 + G_j ----
            Hp = poolH1.tile([P, 18, FD], dt, tag="h1", name="Hp")

            def mkpl(t):
                def f(r, i, j):
                    return t[:, _q(r, i, j)]
                return f

            hp = mkpl(Hp)
            # G4 = c8*I + c9*B  (scalar engine)
            for i in range(3):
                for j in range(3):
                    for r in (0, 1):
                        if r == 0 and i == j:
                            nc.scalar.activation(hp(r, i, j), bpl(r, i, j),
                                                 mybir.ActivationFunctionType.Copy,
                                                 bias=float(cofs[8]),
                                                 scale=float(cofs[9]))
                        else:
                            nc.scalar.mul(hp(r, i, j), bpl(r, i, j), float(cofs[9]))

            cur, curpl = Hp, hp
            other = None
            for step, j in enumerate((3, 2, 1, 0)):
                if step == 0:
                    nxt = poolH2.tile([P, 18, FD], dt, tag="h2", name="Hx")
                elif step % 2 == 1:
                    nxt = poolH1.tile([P, 18, FD], dt, tag="h1", name="Hx")
                else:
                    nxt = poolH2.tile([P, 18, FD], dt, tag="h2", name="Hx")
                nxtpl = mkpl(nxt)
                cmatmul(nxtpl, curpl, b2pl,
                        fold=(float(cofs[2 * j + 1]), float(cofs[2 * j])), foldB=bpl)
                cur, curpl = nxt, nxtpl

            # ---- U load (into B's slot; B dead after last fold) ----
            Ut = poolB.tile([P, 18, FD], dt, tag="bu", name="Bt")
            nc.sync.dma_start(out=Ut[:], in_=Uin[:])
            upl = mkpl(Ut)

            # ---- s squarings (ping-pong H1/H2; cur is in h1 after 4 horner steps) ----
            for k in range(s):
                if k % 2 == 0:
                    nxt = poolH2.tile([P, 18, FD], dt, tag="h2", name="Hx")
                else:
                    nxt = poolH1.tile([P, 18, FD], dt, tag="h1", name="Hx")
                nxtpl = mkpl(nxt)
                cmatmul(nxtpl, curpl, curpl)
                cur, curpl = nxt, nxtpl

            # ---- final OUT = H*U (into PB2 slot) ----
            Ot = poolPB2.tile([P, 18, FD], dt, tag="pb2", name="Ot")
            opl = mkpl(Ot)
            cmatmul(opl, curpl, upl)
            nc.sync.dma_start(
                out=Out[:],
                in_=Ot[:].rearrange("p q (u f) -> p q u f", f=FS))

    nc.finalize()
    return nc


def _get_program(vr, vi, s):
    key = (s, vr.tobytes(), vi.tobytes())
    if key not in _CACHE:
        _CACHE[key] = _build_program(vr, vi, s)
    return _CACHE[key]


def kernel(weights, W, U):
    from concourse.bass_utils import run_bass_kernel_spmd

    out_dtype = np.result_type(np.complex64, weights.dtype, W.dtype, U.dtype)
    w = np.asarray(weights, dtype=np.complex128)
    Wc = np.ascontiguousarray(np.asarray(W, dtype=np.complex128).reshape(NF, V, 3, 3))
    Uc = np.ascontiguousarray(np.asarray(U, dtype=np.complex128).reshape(NDIR, V, 3, 3))

    # choose s from max Frobenius norm of A = (i/2) sum_m w_um * P_m
    Wre = Wc.real.astype(np.float32)
    Wim = Wc.imag.astype(np.float32)
    # P planes on host (cheap, for norm only): tlh2 = 2*tlh
    # pre_ij = Wim_ij + Wim_ji (i!=j); pre_ii = 2Wim_ii - (2/3) sum_k Wim_kk
    # pim_ij = Wre_ji - Wre_ij
    trc = Wim[..., 0, 0] + Wim[..., 1, 1] + Wim[..., 2, 2]
    Pre = Wim + np.swapaxes(Wim, -1, -2)
    for i in range(3):
        Pre[..., i, i] = 2.0 * Wim[..., i, i] - (2.0 / 3.0) * trc
    Pim = np.swapaxes(Wre, -1, -2) - Wre
    # A = (i/2) sum w P  -> Afro^2 = (1/4) sum_ij |sum_m w_um P_m,ij|^2
    Pc = (Pre + 1j * Pim).astype(np.complex64)
    Au = np.einsum('um,msij->usij', w.astype(np.complex64), Pc) * 0.5j
    fro2 = (np.abs(Au) ** 2).sum(axis=(-2, -1))
    maxfro = float(np.sqrt(fro2.max()))
    s = max(0, math.ceil(math.log2(max(maxfro, 1e-6) / THETA)))
    sigma = 2.0 ** -s
    v = 0.5j * sigma * w  # [u, m] complex
    vr = np.ascontiguousarray(v.real.astype(np.float64))
    vi = np.ascontiguousarray(v.imag.astype(np.float64))

    nc = _get_program(vr, vi, s)

    # pack per-core inputs
    in_maps = []
    for k in range(NCORES):
        sl = slice(k * SPC, (k + 1) * SPC)
        y = Wc[:, sl].reshape(NF, P, FS, 3, 3)
        yr = np.stack([y.real, y.imag], axis=1)          # [m,2,p,f,i,j]
        Wp = np.ascontiguousarray(
            yr.transpose(2, 0, 1, 4, 5, 3).reshape(P, NF, 18, FS), dtype=np.float32)
        z = Uc[:, sl].reshape(NDIR, P, FS, 3, 3)
        zr = np.stack([z.real, z.imag], axis=1)          # [u,2,p,f,i,j]
        Up = np.ascontiguousarray(
            zr.transpose(2, 1, 4, 5, 0, 3).reshape(P, 18, NDIR, FS), dtype=np.float32)
        in_maps.append({"Win": Wp, "Uin": Up})

    res = run_bass_kernel_spmd(nc, in_maps, core_ids=list(range(NCORES)))

    out = np.empty((NDIR, V, 3, 3), dtype=np.complex64)
    for k in range(NCORES):
        o = res.results[k]["Out"].reshape(P, 2, 3, 3, NDIR, FS)
        oc = o[:, 0] + 1j * o[:, 1]                      # [p,i,j,u,f]
        out[:, k * SPC:(k + 1) * SPC] = (
            oc.transpose(3, 0, 4, 1, 2).reshape(NDIR, SPC, 3, 3))
    return out.reshape((NDIR,) + LAT + (3, 3)).astype(out_dtype)
